# revision 3
# baseline (speedup 1.0000x reference)
"""Trainium2 Bass kernel for the additive-attention problem (V3).

reference math:
    rec[b,h]    = sum_r rnn_state[b,r] * W_rec[h,r]
    scores[t,b] = sum_h tanh(enc[t,b,h] + rec[b,h]) * w_score[h] + b_score + mask[t,b]
    out         = softmax(scores, axis=t)          # (T, B) float32

Sharding: data-parallel over B across 8 cores (BL=4 batch columns per core).
Softmax is over T (core-local) -> no collectives.

V3 layout: h on partitions (host ships encT = enc.transpose(1,2,0) as fp16,
halving HBM traffic vs f32; ~47us DMA floor at ~358GB/s/core).  Per (b, hc)
tile [p=h%128, f=t (4096)]:
  - DMA fp16 tile (1MB, 8KB contiguous rows)
  - ACT: Y = tanh(enc + rec) in ONE op - rec[b, hc*128+p] is a per-partition
    bias AP (the broadcast add costs nothing on VectorE)
  - PE: score reduction over h via matmuls: lhsT = Y[:, tc*128:(tc+1)*128]
    (stationary, fp16), rhs = w_score column for hc -> psum col [t%128, 1].
    Partials per hc are kept in separate psum columns (start=stop=True, no
    interleaved accumulation groups); VectorE sums the 4 hc-partials.
Scores land as [p=t%128, f=(tc,b)] = the baseline softmax-tail layout:
mask add, exp, PE transpose, row sums, block-mask matmul broadcast of per-b
totals, reciprocal, scale, DMA out as (BL,T) contiguous rows.
b_score cancels in softmax; no max-subtraction needed (|scores| <~ 25).
fp16 (not bf16) everywhere on the elementwise path for extra mantissa.
ACT is the expected bottleneck: 16 tanh x (4096+222)cyc @ 1.2GHz ~= 58us.
"""

import numpy as np

T, B, H, R = 4096, 32, 512, 512
NCORES = 8
BL = B // NCORES          # 4 local batch columns
HC = H // 128             # 4 h-chunks
RC = R // 128             # 4 r-chunks
NTC = T // 128            # 32 t-chunks of 128

_GRAPH = None


def _build_graph():
    import concourse.bass as bass
    import concourse.tile as tile
    from concourse import bacc, mybir
    from concourse.masks import make_identity

    f32 = mybir.dt.float32
    f16 = mybir.dt.float16
    nc = bacc.Bacc()

    encT = nc.declare_dram_parameter("encT", [BL, H, T], f16, isOutput=False)
    maskd = nc.declare_dram_parameter("maskd", [T, BL], f32, isOutput=False)
    rnnT = nc.declare_dram_parameter("rnnT", [R, BL], f16, isOutput=False)
    wrecT = nc.declare_dram_parameter("wrecT", [R, H], f16, isOutput=False)
    wcold = nc.declare_dram_parameter("wcold", [128, HC], f16, isOutput=False)
    m4d = nc.declare_dram_parameter("m4", [128, 128], f32, isOutput=False)
    out = nc.declare_dram_parameter("out", [BL, T], f32, isOutput=True)

    with tile.TileContext(nc) as tc:
        with (
            tc.tile_pool(name="singles", bufs=1) as singles,
            tc.tile_pool(name="xpool", bufs=4) as xpool,
            tc.tile_pool(name="ypool", bufs=6) as ypool,
            tc.tile_pool(name="spool", bufs=2, space="PSUM") as spool,
        ):
            # ---------- small loads (sync/HWDGE first => FIFO before enc) ----
            rnn_sb = singles.tile([128, RC, BL], f16)
            nc.sync.dma_start(
                out=rnn_sb[:], in_=rnnT.rearrange("(rc p) b -> p rc b", p=128)
            )
            wrec_sb = singles.tile([128, RC, H], f16)
            nc.sync.dma_start(
                out=wrec_sb[:], in_=wrecT.rearrange("(rc p) h -> p rc h", p=128)
            )
            wcol = singles.tile([128, HC], f16)
            nc.sync.dma_start(out=wcol[:], in_=wcold[:])
            m4 = singles.tile([128, 128], f32)
            nc.sync.dma_start(out=m4[:], in_=m4d[:])
            mask_sb = singles.tile([128, NTC, BL], f32)
            nc.sync.dma_start(
                out=mask_sb[:], in_=maskd.rearrange("(tc p) b -> p tc b", p=128)
            )
            ident = singles.tile([128, 128], f32)
            make_identity(nc, ident[:])

            # rec.T[h, b] = sum_r W_rec[h, r] * rnn[b, r], per h-chunk
            rec_sb = singles.tile([128, HC, BL], f32)
            for hc in range(HC):
                rp = spool.tile([128, BL], f32, tag="rec")
                for rc in range(RC):
                    nc.tensor.matmul(
                        rp[:],
                        lhsT=wrec_sb[:, rc, hc * 128 : (hc + 1) * 128],
                        rhs=rnn_sb[:, rc, :],
                        start=(rc == 0),
                        stop=(rc == RC - 1),
                    )
                nc.vector.tensor_copy(out=rec_sb[:, hc, :], in_=rp[:])

            scores = singles.tile([128, NTC, BL], f32)
            encv = encT.rearrange("b (hc p) t -> b hc p t", p=128)

            # ---------- main loop: per (b, hc) tile [128, T] ----------
            for b in range(BL):
                Pp = spool.tile([128, HC * NTC], f32, tag="partials")
                for hc in range(HC):
                    X = xpool.tile([128, T], f16)
                    nc.sync.dma_start(out=X[:], in_=encv[b, hc])
                    Y = ypool.tile([128, T], f16)
                    nc.scalar.activation(
                        out=Y[:],
                        in_=X[:],
                        func=mybir.ActivationFunctionType.Tanh,
                        bias=rec_sb[:, hc, b : b + 1],
                    )
                    for tcng in range(NTC):
                        c = hc * NTC + tcng
                        nc.tensor.matmul(
                            Pp[:, c : c + 1],
                            lhsT=Y[:, tcng * 128 : (tcng + 1) * 128],
                            rhs=wcol[:, hc : hc + 1],
                            start=True,
                            stop=True,
                        )
                # combine the 4 hc partials -> scores[:, :, b]
                # (never two PSUM operands in one DVE op - single PSUM rd port)
                acc = ypool.tile([128, NTC], f32, tag="comb")
                nc.vector.tensor_copy(out=acc[:], in_=Pp[:, 0:NTC])
                nc.vector.tensor_add(
                    out=acc[:], in0=acc[:], in1=Pp[:, NTC : 2 * NTC]
                )
                nc.vector.tensor_add(
                    out=acc[:], in0=acc[:], in1=Pp[:, 2 * NTC : 3 * NTC]
                )
                nc.vector.tensor_add(
                    out=scores[:, :, b],
                    in0=acc[:],
                    in1=Pp[:, 3 * NTC : 4 * NTC],
                )

            # ---------- mask, exp, softmax normalization, output ----------
            sflat = scores[:].rearrange("p tc b -> p (tc b)")
            nc.vector.tensor_add(
                out=sflat,
                in0=sflat,
                in1=mask_sb[:].rearrange("p tc b -> p (tc b)"),
            )
            E = singles.tile([128, 128], f32)
            nc.scalar.activation(
                out=E[:], in_=sflat,
                func=mybir.ActivationFunctionType.Exp,
            )
            # transpose: (p=t%128, f=(tc,b)) -> (p=(tc,b), f=t%128)
            attT = spool.tile([128, 128], f32, tag="attT")
            nc.tensor.transpose(out=attT[:], in_=E[:], identity=ident[:])
            row_sums = singles.tile([128, 1], f32)
            nc.vector.tensor_reduce(
                out=row_sums[:], in_=attT[:], axis=mybir.AxisListType.X,
                op=mybir.AluOpType.add,
            )
            denom = spool.tile([128, 1], f32, tag="denom")
            nc.tensor.matmul(
                denom[:], lhsT=m4[:], rhs=row_sums[:], start=True, stop=True
            )
            recip = singles.tile([128, 1], f32)
            nc.vector.reciprocal(out=recip[:], in_=denom[:])
            att_out = singles.tile([128, 128], f32)
            nc.vector.tensor_scalar_mul(
                out=att_out[:], in0=attT[:], scalar1=recip[:]
            )
            # partition p = (tc, b) holds 128 contiguous t values for col b
            nc.sync.dma_start(
                out=out.rearrange("b (tc tp) -> tc b tp", tp=128),
                in_=att_out[:],
            )

    nc.compile()
    return nc


def _get_graph():
    global _GRAPH
    if _GRAPH is None:
        _GRAPH = _build_graph()
    return _GRAPH


def make_in_maps(enc, mask, rnn_state, W_rec, w_score):
    enc16 = np.asarray(enc, dtype=np.float16)
    # [T, B, H] -> [B, H, T]
    encT_full = np.ascontiguousarray(enc16.transpose(1, 2, 0))
    wrecT = np.ascontiguousarray(W_rec.T.astype(np.float16))
    wcol = np.ascontiguousarray(
        np.asarray(w_score, dtype=np.float16).reshape(HC, 128).T
    )
    m4 = (np.arange(128)[:, None] % BL == np.arange(128)[None, :] % BL).astype(
        np.float32
    )
    in_maps = []
    for c in range(NCORES):
        sl = slice(c * BL, (c + 1) * BL)
        in_maps.append(
            {
                "encT": np.ascontiguousarray(encT_full[sl]),
                "maskd": np.ascontiguousarray(mask[:, sl].astype(np.float32)),
                "rnnT": np.ascontiguousarray(rnn_state[sl].T.astype(np.float16)),
                "wrecT": wrecT,
                "wcold": wcol,
                "m4": m4,
            }
        )
    return in_maps


def kernel(
    encoded_contribution,
    mask,
    rnn_state,
    prev_att_weights,
    W_rec,
    w_score,
    b_score,
):
    from concourse.bass_utils import run_bass_kernel_spmd

    nc = _get_graph()
    in_maps = make_in_maps(
        np.asarray(encoded_contribution),
        np.asarray(mask),
        np.asarray(rnn_state),
        np.asarray(W_rec),
        np.asarray(w_score),
    )
    res = run_bass_kernel_spmd(nc, in_maps, list(range(NCORES)))
    outs = [np.asarray(res.results[c]["out"]) for c in range(NCORES)]
    return np.concatenate([o.T for o in outs], axis=1).astype(np.float32)


# revision 4
# speedup vs baseline: 1.0259x; 1.0259x over previous
"""Trainium2 Bass kernel for the additive-attention problem (V3.1).

reference math:
    rec[b,h]    = sum_r rnn_state[b,r] * W_rec[h,r]
    scores[t,b] = sum_h tanh(enc[t,b,h] + rec[b,h]) * w_score[h] + b_score + mask[t,b]
    out         = softmax(scores, axis=t)          # (T, B) float32

Sharding: data-parallel over B across 8 cores (BL=4 batch columns per core).
Softmax is over T (core-local) -> no collectives.

Layout: h on partitions (host ships encT = enc.transpose(1,2,0) as fp16,
halving HBM traffic vs f32; ~47us DMA floor at ~358GB/s/core).  Per (b, hc)
tile [p=h%128, f=t (4096)]:
  - DMA fp16 tile (1MB, 8KB contiguous rows)
  - ACT: Y = tanh(enc + rec) in ONE op - rec[b, hc*128+p] is a per-partition
    bias AP (the broadcast add costs nothing on VectorE)
  - PE: score reduction over h via matmuls: lhsT = Y[:, tc*128:(tc+1)*128]
    (stationary, fp16), rhs = w_score column for hc -> psum col [t%128, 1].
    Partials per hc go to separate psum columns (start=stop=True, no
    interleaved accumulation groups); VectorE sums the 4 hc-partials
    (chained, never 2 PSUM operands in one DVE op).
Scores land as [p=t%128, f=(tc,b)]: mask add, exp, PE transpose, row sums,
block-mask matmul broadcast of per-b totals, reciprocal, scale, DMA out as
(BL,T) contiguous rows.  b_score cancels in softmax; no max-subtraction
needed (|scores| <~ 25).  fp16 everywhere on the elementwise path.

ACT is the bottleneck (16 x (4096+222)cyc @ 1.2GHz ~= 58us back-to-back,
measured gap=0).  V3.1 trims the edges of the critical path:
  - dummy tanh on the identity tile hoists ACT_TABLE_LOAD to t~=0
  - wrec shipped per-hc so the hc=0 rec bias is ready ~10us (not 13.5)
  - first enc tile split in 2 half-DMAs/tanhs (first tanh ~10.5us, not 15.5)
  - last tile split in 2 so the final PE pass + softmax tail overlap
  - m4/mask loads issued from idle GpSimd (SWDGE), enc stream owns sync
"""

import numpy as np

T, B, H, R = 4096, 32, 512, 512
NCORES = 8
BL = B // NCORES          # 4 local batch columns
HC = H // 128             # 4 h-chunks
RC = R // 128             # 4 r-chunks
NTC = T // 128            # 32 t-chunks of 128

_GRAPH = None


def _build_graph():
    import concourse.bass as bass
    import concourse.tile as tile
    from concourse import bacc, mybir
    from concourse.masks import make_identity

    f32 = mybir.dt.float32
    f16 = mybir.dt.float16
    nc = bacc.Bacc()

    encT = nc.declare_dram_parameter("encT", [BL, H, T], f16, isOutput=False)
    maskd = nc.declare_dram_parameter("maskd", [T, BL], f32, isOutput=False)
    rnnT = nc.declare_dram_parameter("rnnT", [R, BL], f16, isOutput=False)
    wrecd = nc.declare_dram_parameter("wrecd", [HC, R, 128], f16, isOutput=False)
    wcold = nc.declare_dram_parameter("wcold", [128, HC], f16, isOutput=False)
    m4d = nc.declare_dram_parameter("m4", [128, 128], f32, isOutput=False)
    out = nc.declare_dram_parameter("out", [BL, T], f32, isOutput=True)

    with tile.TileContext(nc) as tc:
        with (
            tc.tile_pool(name="singles", bufs=1) as singles,
            tc.tile_pool(name="xpool", bufs=4) as xpool,
            tc.tile_pool(name="ypool", bufs=6) as ypool,
            tc.tile_pool(name="spool", bufs=2, space="PSUM") as spool,
        ):
            ident = singles.tile([128, 128], f32)
            make_identity(nc, ident[:])
            # dummy tanh: forces ACT_TABLE_LOAD at t~=0 (no DMA dependency)
            dummy = singles.tile([128, 1], f32)
            nc.scalar.activation(
                out=dummy[:], in_=ident[:, 0:1],
                func=mybir.ActivationFunctionType.Tanh,
            )

            # ---- small loads on sync/HWDGE in latency-criticality order ----
            rnn_sb = singles.tile([128, RC, BL], f16)
            nc.sync.dma_start(
                out=rnn_sb[:], in_=rnnT.rearrange("(rc p) b -> p rc b", p=128)
            )
            wrecv = wrecd.rearrange("c (rc p) w -> c p rc w", p=128)
            wrec_sb = singles.tile([128, HC, RC, 128], f16)
            for hc in range(HC):
                nc.sync.dma_start(out=wrec_sb[:, hc], in_=wrecv[hc])
            wcol = singles.tile([128, HC], f16)
            nc.sync.dma_start(out=wcol[:], in_=wcold[:])
            # not needed until the softmax tail -> keep off the sync queue
            m4 = singles.tile([128, 128], f32)
            nc.gpsimd.dma_start(out=m4[:], in_=m4d[:])
            mask_sb = singles.tile([128, NTC, BL], f32)
            nc.gpsimd.dma_start(
                out=mask_sb[:], in_=maskd.rearrange("(tc p) b -> p tc b", p=128)
            )

            # rec.T[h, b] = sum_r W_rec[h, r] * rnn[b, r], per h-chunk
            rec_sb = singles.tile([128, HC, BL], f32)
            for hc in range(HC):
                rp = spool.tile([128, BL], f32, tag="rec")
                for rc in range(RC):
                    nc.tensor.matmul(
                        rp[:],
                        lhsT=wrec_sb[:, hc, rc],
                        rhs=rnn_sb[:, rc, :],
                        start=(rc == 0),
                        stop=(rc == RC - 1),
                    )
                nc.vector.tensor_copy(out=rec_sb[:, hc, :], in_=rp[:])

            scores = singles.tile([128, NTC, BL], f32)
            encv = encT.rearrange("b (hc p) t -> b hc p t", p=128)

            # ---------- main loop: per (b, hc) tile [128, T] ----------
            for b in range(BL):
                Pp = spool.tile([128, HC * NTC], f32, tag="partials")
                for hc in range(HC):
                    split = (b == 0 and hc == 0) or (b == BL - 1 and hc == HC - 1)
                    nsp = 2 if split else 1
                    X = xpool.tile([128, T], f16)
                    Y = ypool.tile([128, T], f16)
                    for s in range(nsp):
                        sl = slice(s * T // nsp, (s + 1) * T // nsp)
                        nc.sync.dma_start(out=X[:, sl], in_=encv[b, hc, :, sl])
                        nc.scalar.activation(
                            out=Y[:, sl],
                            in_=X[:, sl],
                            func=mybir.ActivationFunctionType.Tanh,
                            bias=rec_sb[:, hc, b : b + 1],
                        )
                        for tcng in range(s * NTC // nsp, (s + 1) * NTC // nsp):
                            c = hc * NTC + tcng
                            nc.tensor.matmul(
                                Pp[:, c : c + 1],
                                lhsT=Y[:, tcng * 128 : (tcng + 1) * 128],
                                rhs=wcol[:, hc : hc + 1],
                                start=True,
                                stop=True,
                            )
                # combine the 4 hc partials -> scores[:, :, b]
                # (never two PSUM operands in one DVE op - single PSUM rd port)
                acc = ypool.tile([128, NTC], f32, tag="comb")
                nc.vector.tensor_copy(out=acc[:], in_=Pp[:, 0:NTC])
                nc.vector.tensor_add(
                    out=acc[:], in0=acc[:], in1=Pp[:, NTC : 2 * NTC]
                )
                nc.vector.tensor_add(
                    out=acc[:], in0=acc[:], in1=Pp[:, 2 * NTC : 3 * NTC]
                )
                nc.vector.tensor_add(
                    out=scores[:, :, b],
                    in0=acc[:],
                    in1=Pp[:, 3 * NTC : 4 * NTC],
                )

            # ---------- mask, exp, softmax normalization, output ----------
            sflat = scores[:].rearrange("p tc b -> p (tc b)")
            nc.vector.tensor_add(
                out=sflat,
                in0=sflat,
                in1=mask_sb[:].rearrange("p tc b -> p (tc b)"),
            )
            E = singles.tile([128, 128], f32)
            nc.scalar.activation(
                out=E[:], in_=sflat,
                func=mybir.ActivationFunctionType.Exp,
            )
            # transpose: (p=t%128, f=(tc,b)) -> (p=(tc,b), f=t%128)
            attT = spool.tile([128, 128], f32, tag="attT")
            nc.tensor.transpose(out=attT[:], in_=E[:], identity=ident[:])
            row_sums = singles.tile([128, 1], f32)
            nc.vector.tensor_reduce(
                out=row_sums[:], in_=attT[:], axis=mybir.AxisListType.X,
                op=mybir.AluOpType.add,
            )
            denom = spool.tile([128, 1], f32, tag="denom")
            nc.tensor.matmul(
                denom[:], lhsT=m4[:], rhs=row_sums[:], start=True, stop=True
            )
            recip = singles.tile([128, 1], f32)
            nc.vector.reciprocal(out=recip[:], in_=denom[:])
            att_out = singles.tile([128, 128], f32)
            nc.vector.tensor_scalar_mul(
                out=att_out[:], in0=attT[:], scalar1=recip[:]
            )
            # partition p = (tc, b) holds 128 contiguous t values for col b
            nc.sync.dma_start(
                out=out.rearrange("b (tc tp) -> tc b tp", tp=128),
                in_=att_out[:],
            )

    nc.compile()
    return nc


def _get_graph():
    global _GRAPH
    if _GRAPH is None:
        _GRAPH = _build_graph()
    return _GRAPH


def make_in_maps(enc, mask, rnn_state, W_rec, w_score):
    enc16 = np.asarray(enc, dtype=np.float16)
    # [T, B, H] -> [B, H, T]
    encT_full = np.ascontiguousarray(enc16.transpose(1, 2, 0))
    # [HC, R, 128]: per-hc chunks of W_rec.T
    wrecd = np.ascontiguousarray(
        W_rec.T.astype(np.float16).reshape(R, HC, 128).transpose(1, 0, 2)
    )
    wcol = np.ascontiguousarray(
        np.asarray(w_score, dtype=np.float16).reshape(HC, 128).T
    )
    m4 = (np.arange(128)[:, None] % BL == np.arange(128)[None, :] % BL).astype(
        np.float32
    )
    in_maps = []
    for c in range(NCORES):
        sl = slice(c * BL, (c + 1) * BL)
        in_maps.append(
            {
                "encT": np.ascontiguousarray(encT_full[sl]),
                "maskd": np.ascontiguousarray(mask[:, sl].astype(np.float32)),
                "rnnT": np.ascontiguousarray(rnn_state[sl].T.astype(np.float16)),
                "wrecd": wrecd,
                "wcold": wcol,
                "m4": m4,
            }
        )
    return in_maps


def kernel(
    encoded_contribution,
    mask,
    rnn_state,
    prev_att_weights,
    W_rec,
    w_score,
    b_score,
):
    from concourse.bass_utils import run_bass_kernel_spmd

    nc = _get_graph()
    in_maps = make_in_maps(
        np.asarray(encoded_contribution),
        np.asarray(mask),
        np.asarray(rnn_state),
        np.asarray(W_rec),
        np.asarray(w_score),
    )
    res = run_bass_kernel_spmd(nc, in_maps, list(range(NCORES)))
    outs = [np.asarray(res.results[c]["out"]) for c in range(NCORES)]
    return np.concatenate([o.T for o in outs], axis=1).astype(np.float32)
